# revision 1
# baseline (speedup 1.0000x reference)
"""HAN layer (4-metapath GAT + semantic attention) for Trainium2, 8 NeuronCores.

Sharding: core c handles metapath m = c % 4, node-half h = c // 4
(N=30000 nodes -> two halves of 15000, padded to 15104 = 118 * 128).
Each core computes its feature projection feat = hs[m][half] @ W[m] on the
tensor engine ([15104,128] @ [128,256] as 118 PSUM-tile matmuls).
The data-dependent edge phase (edge softmax + neighborhood aggregation) and
the tiny semantic-attention reduction run on the host over the
device-computed projections.
"""
import sys
import numpy as np

sys.path.insert(0, "/opt/trn_rl_repo")

N, E, IN, H, D = 30000, 300000, 128, 4, 64
HD = H * D                      # 256
M = 4                           # metapaths
NCORES = 8
HALF = N // 2                   # 15000
HPAD = 15104                    # 118 * 128
P = 128
NT = HPAD // P                  # 118 tiles per core
NEG_ATTN = 0.2
NEG_ACT = 0.01


def _build_bass():
    import concourse.bacc as bacc
    import concourse.tile as tile
    from concourse import mybir
    from contextlib import ExitStack

    nc = bacc.Bacc()
    hsT = nc.declare_dram_parameter("hsT", (P, HPAD), mybir.dt.float32, isOutput=False)
    Wm = nc.declare_dram_parameter("Wm", (P, HD), mybir.dt.float32, isOutput=False)
    feat = nc.declare_dram_parameter("feat", (HPAD, HD), mybir.dt.float32, isOutput=True)

    with tile.TileContext(nc) as tc, ExitStack() as ctx:
        sb = ctx.enter_context(tc.tile_pool(name="sb", bufs=3))
        ps = ctx.enter_context(tc.tile_pool(name="ps", bufs=2, space="PSUM"))
        w_sb = sb.tile([P, HD], mybir.dt.float32, tag="w")
        nc.sync.dma_start(out=w_sb[:], in_=Wm[:, :])
        for t in range(NT):
            lhsT = sb.tile([P, P], mybir.dt.float32, tag="lhsT")
            nc.sync.dma_start(out=lhsT[:], in_=hsT[:, t * P:(t + 1) * P])
            acc = ps.tile([P, HD], mybir.dt.float32, space="PSUM", tag="acc")
            nc.tensor.matmul(out=acc[:], lhsT=lhsT[:], rhs=w_sb[:],
                             start=True, stop=True)
            stg = sb.tile([P, HD], mybir.dt.float32, tag="stg")
            nc.scalar.copy(out=stg[:], in_=acc[:])
            nc.sync.dma_start(out=feat[t * P:(t + 1) * P, :], in_=stg[:])
    nc.compile()
    return nc


def _run_device(hs, W):
    """Returns feats[m] = hs[m] @ W[m] as [N, HD], computed on 8 neuron cores."""
    from concourse.bass_utils import run_bass_kernel_spmd

    nc = _build_bass()
    in_maps = []
    for c in range(NCORES):
        m, h = c % M, c // M
        hs_half = hs[m][h * HALF:(h + 1) * HALF]            # [15000, 128]
        hsT = np.zeros((P, HPAD), np.float32)
        hsT[:, :HALF] = hs_half.T
        in_maps.append({"hsT": np.ascontiguousarray(hsT),
                        "Wm": np.ascontiguousarray(W[m])})
    res = run_bass_kernel_spmd(nc, in_maps, list(range(NCORES)))
    feats = []
    for m in range(M):
        top = res.results[m]["feat"][:HALF]
        bot = res.results[m + 4]["feat"][:HALF]
        feats.append(np.concatenate([top, bot], axis=0))    # [N, HD]
    return feats, res


def _gat_edge_phase(featm, src, dst, al, ar, b):
    """Edge softmax + aggregation, numerically identical to the reference
    (alpha = ex/sum(ex) is invariant to the max-shift; |e| < ~3 so exp is safe)."""
    f = featm.reshape(N, H, D)
    el = (f * al).sum(-1)                                   # [N, H]
    er = (f * ar).sum(-1)
    e = el[src] + er[dst]
    e = np.where(e > 0, e, NEG_ATTN * e)                    # leaky_relu 0.2
    ex = np.exp(e)                                          # [E, H]
    order = np.argsort(dst, kind="stable")
    ds = dst[order]
    starts = np.flatnonzero(np.r_[True, ds[1:] != ds[:-1]])
    uniq = ds[starts]
    exs = ex[order]
    den = np.add.reduceat(exs, starts, axis=0)              # [U, H]
    msg = f[src[order]] * exs[:, :, None]                   # [E, H, D]
    sums = np.add.reduceat(msg.reshape(E, HD), starts, axis=0)
    out = np.zeros((N, H, D), np.float32)
    out[uniq] = sums.reshape(-1, H, D) / np.maximum(den, 1e-9)[:, :, None]
    out = out + b.reshape(1, H, D)
    out = np.where(out > 0, out, NEG_ACT * out)             # leaky_relu 0.01
    return out.reshape(N, HD).astype(np.float32)


def _semantic(z, Wp1, bp1, Wp2):
    w = (np.tanh(z @ Wp1 + bp1) @ Wp2).mean(0)              # [2, 1]
    w = w - w.max()
    beta = np.exp(w) / np.exp(w).sum()
    return (beta[None] * z).sum(1)


def kernel(hs, src, dst, W, attn_l, attn_r, bias, Wp1, bp1, Wp2):
    hs = np.asarray(hs, np.float32)
    src = np.asarray(src)
    dst = np.asarray(dst)
    W = np.asarray(W, np.float32)

    feats, _ = _run_device(hs, W)

    outs = []
    for m in range(M):
        outs.append(_gat_edge_phase(feats[m], src[m].astype(np.int64),
                                    dst[m].astype(np.int64),
                                    np.asarray(attn_l[m]), np.asarray(attn_r[m]),
                                    np.asarray(bias[m])))
    Wp1 = np.asarray(Wp1); bp1 = np.asarray(bp1); Wp2 = np.asarray(Wp2)
    lnc = _semantic(np.stack([outs[1], outs[2]], axis=1), Wp1, bp1, Wp2)
    dis = _semantic(np.stack([outs[0], outs[3]], axis=1), Wp1, bp1, Wp2)
    return np.stack([lnc, dis]).astype(np.float32)



# revision 22
# speedup vs baseline: 2954.5898x; 2954.5898x over previous
"""HAN layer (4-metapath GAT + semantic attention) on Trainium2, 8 NeuronCores.

Sharding: core c -> metapath m = c % 4, node-half h = c // 4.
Per metapath, nodes are sorted by in-degree (ascending, 208 dummy slots first)
into 236 tiles of 128; half h takes tiles g = 2i + h (118 tiles/core), so both
halves see near-identical degree profiles and one SPMD program (common slot
schedule S[i] = max over the 8 shards) fits all cores with ~no padding waste.

Device pipeline per core:
  Phase A: feat_ext = hs @ [W | W@Al | W@Ar] on the tensor engine ->
           gather table [30081 x 768B] rows of (256 feat bf16 | el,er f32),
           row 30080 is the pad row (el = -300 so exp(leaky(el+er)) ~= 0).
  Phase B: per node tile, one dma_gather pulls S slots x 128 rows (src
           neighbors + a self slot supplying er[dst]); DVE/ACT compute
           leaky->exp->softmax-normalize->aggregate; per-tile semantic
           partials tanh(z@Wp1+bp1)@Wp2 accumulate on PE/DVE.
Host: index prep (argsort/CSR/padding), final beta softmax + weighted sum.
"""
import sys
import numpy as np

sys.path.insert(0, "/opt/trn_rl_repo")

N, E, IN, H, D = 30000, 300000, 128, 4, 64
HD = H * D                      # 256
M = 4
NCORES = 8
P = 128
NPAD = 30208                    # 236 * 128 node slots per metapath
NTG = NPAD // P                 # 236 global node tiles
NTB = NTG // 2                  # 118 tiles per core
NDUMMY = NPAD - N               # 208
TROWS = 30080                   # 235*128 real table rows (>= N)
NTA = TROWS // P                # 235 phase-A tiles
PADIDX = 30080                  # pad row index in table
TBL_ROWS = TROWS + 1            # 30081
ROWB = 384                      # bf16 elems per table row (768 B)
OUTROWS = NTB * P               # 15104
NEG_ATTN = 0.2
NEG_ACT = 0.01
EL_NEG = -300.0                 # pad-row el: exp(0.2*(-300+er)) ~ 1e-26
SMAXG = 8                       # dma_gather dies above 1024 idxs => <=8 slots

_BUILD_CACHE = {}


# ---------------------------------------------------------------- host: indices
def _prep_metapath(src_m, dst_m):
    """Degree-sorted slot layout + padded neighbor matrix for one metapath."""
    deg = np.bincount(dst_m, minlength=N)
    order_nodes = np.argsort(deg, kind="stable")          # ascending degree
    slot_nodes = np.concatenate([np.full(NDUMMY, -1, np.int64), order_nodes])
    slot_of = np.empty(N, np.int64)
    slot_of[order_nodes] = np.arange(NDUMMY, NPAD)

    eorder = np.argsort(dst_m, kind="stable")
    ds = dst_m[eorder]
    ss = src_m[eorder]
    starts = np.concatenate([[0], np.cumsum(deg)])[:N]
    within = np.arange(E) - starts[ds]

    maxdeg = int(deg.max())
    A = np.full((NPAD, maxdeg), PADIDX, np.int16)
    A[slot_of[ds], within] = ss.astype(np.int16)

    degs_sorted = np.concatenate([np.zeros(NDUMMY, np.int64), deg[order_nodes]])
    tiledeg = degs_sorted.reshape(NTG, P).max(axis=1)     # [236]
    selfcol = np.where(slot_nodes >= 0, slot_nodes, PADIDX).astype(np.int16)
    return {"A": A, "tiledeg": tiledeg, "slot_nodes": slot_nodes,
            "selfcol": selfcol}


def _common_schedule(preps):
    """S[i] per core-tile rank i (shared across all 8 cores)."""
    S = np.zeros(NTB, np.int64)
    for i in range(NTB):
        md = 0
        for p in preps:
            md = max(md, int(p["tiledeg"][2 * i]), int(p["tiledeg"][2 * i + 1]))
        S[i] = md + 1                                      # + self slot
    return np.maximum(S, 2)


def _core_idxs(prep, h, S):
    """Wrapped int16 index tile [128, IC] for core (metapath prep, half h)."""
    blocks = []
    for i in range(NTB):
        g = 2 * i + h
        rows = slice(g * P, (g + 1) * P)
        Si = int(S[i])
        full = np.full((P, Si), PADIDX, np.int16)
        width = min(Si - 1, prep["A"].shape[1])
        full[:, :width] = prep["A"][rows, :width]
        full[:, Si - 1] = prep["selfcol"][rows]
        for c0 in range(0, Si, SMAXG):                     # <=8-slot chunks
            Sc = min(SMAXG, Si - c0)
            T = full[:, c0:c0 + Sc].T.ravel()              # e = s*128 + p
            blocks.append(T.reshape(8 * Sc, 16).T)         # [16, 8*Sc]
    cols = np.concatenate(blocks, axis=1)
    # idx wrap lives in 16 partitions, replicated into all 8 Q7-core groups
    return np.tile(cols, (8, 1))


# ---------------------------------------------------------------- device build
def _build_bass(S, nb_limit=None, do_edge=True, do_sem=True):
    import concourse.bacc as bacc
    import concourse.tile as tile
    from concourse import mybir
    from concourse.masks import make_identity
    from contextlib import ExitStack

    f32 = mybir.dt.float32
    bf16 = mybir.dt.bfloat16
    i16 = mybir.dt.int16
    Alu = mybir.AluOpType
    Act = mybir.ActivationFunctionType
    IC = int(8 * np.sum(S))

    nc = bacc.Bacc()
    hsT = nc.declare_dram_parameter("hsT", (P, TROWS), bf16, isOutput=False)
    Wx = nc.declare_dram_parameter("Wx", (P, HD + 2 * H), bf16, isOutput=False)
    padrow = nc.declare_dram_parameter("padrow", (1, ROWB), bf16, isOutput=False)
    idxs_d = nc.declare_dram_parameter("idxs", (P, IC), i16, isOutput=False)
    bias_rep = nc.declare_dram_parameter("bias_rep", (P, HD), f32, isOutput=False)
    Wp1_d = nc.declare_dram_parameter("Wp1", (HD, 128), f32, isOutput=False)
    bp1_rep = nc.declare_dram_parameter("bp1_rep", (P, 128), f32, isOutput=False)
    Wp2T_rep = nc.declare_dram_parameter("Wp2T_rep", (P, 128), f32, isOutput=False)
    outbuf = nc.declare_dram_parameter("out", (OUTROWS, HD), f32, isOutput=True)
    s_out = nc.declare_dram_parameter("s_out", (P, 1), f32, isOutput=True)

    with tile.TileContext(nc) as tc, ExitStack() as ctx:
        big = ctx.enter_context(tc.tile_pool(name="big", bufs=1))
        sb = ctx.enter_context(tc.tile_pool(name="sb", bufs=3))
        gt = ctx.enter_context(tc.tile_pool(name="gt", bufs=2))
        pr = ctx.enter_context(tc.tile_pool(name="pr", bufs=2))
        ps = ctx.enter_context(tc.tile_pool(name="ps", bufs=2, space="PSUM"))
        dram = ctx.enter_context(tc.tile_pool(name="dram", bufs=1, space="DRAM"))

        table = dram.tile([TBL_ROWS, ROWB], bf16, tag="table")

        # ---- resident loads
        hsT_sb = big.tile([P, TROWS], bf16, tag="hsT")
        # single >= ~60KB/partition DMAs fail (NRT unrecoverable); chunk it
        for q in range(4):
            lo, hi = q * (TROWS // 4), (q + 1) * (TROWS // 4)
            nc.sync.dma_start(out=hsT_sb[:, lo:hi], in_=hsT[:, lo:hi])
        Wx_sb = big.tile([P, HD + 2 * H], bf16, tag="Wx")
        nc.sync.dma_start(out=Wx_sb[:], in_=Wx[:, :])
        idx_sb = big.tile([P, IC], i16, tag="idx")
        nc.sync.dma_start(out=idx_sb[:], in_=idxs_d[:, :])
        bias_sb = big.tile([P, HD], f32, tag="bias")
        nc.sync.dma_start(out=bias_sb[:], in_=bias_rep[:, :])
        Wp1a_sb = big.tile([P, 128], f32, tag="wp1a")
        nc.sync.dma_start(out=Wp1a_sb[:], in_=Wp1_d[0:128, :])
        Wp1b_sb = big.tile([P, 128], f32, tag="wp1b")
        nc.sync.dma_start(out=Wp1b_sb[:], in_=Wp1_d[128:256, :])
        bp1_sb = big.tile([P, 128], f32, tag="bp1")
        nc.sync.dma_start(out=bp1_sb[:], in_=bp1_rep[:, :])
        Wp2_sb = big.tile([P, 128], f32, tag="wp2")
        nc.sync.dma_start(out=Wp2_sb[:], in_=Wp2T_rep[:, :])
        ident = big.tile([P, P], f32, tag="ident")
        make_identity(nc, ident[:])
        if do_edge and do_sem:
            scols = big.tile([P, NTB], f32, tag="scols")
        else:
            scols = None

        pr_sb = sb.tile([1, ROWB], bf16, tag="padrow")
        nc.sync.dma_start(out=pr_sb[:], in_=padrow[:, :])
        nc.sync.dma_start(out=table[TROWS:TROWS + 1, :], in_=pr_sb[:])

        # ---- phase A: build gather table
        for t in range(NTA):
            psf = ps.tile([P, HD + 2 * H], f32, space="PSUM", tag="psf")
            nc.tensor.matmul(out=psf[:], lhsT=hsT_sb[:, t * P:(t + 1) * P],
                             rhs=Wx_sb[:], start=True, stop=True)
            stage = sb.tile([P, ROWB], bf16, tag="stage")
            nc.vector.tensor_copy(out=stage[:, :HD], in_=psf[:, :HD])
            stage_f = stage[:].bitcast(f32)                 # [P, 192]
            nc.vector.tensor_copy(out=stage_f[:, 128:128 + 2 * H],
                                  in_=psf[:, HD:HD + 2 * H])
            nc.sync.dma_start(out=table[t * P:(t + 1) * P, :], in_=stage[:])

        # ---- phase B: gather + edge softmax + aggregate + semantic partials
        off = 0
        nb = NTB if nb_limit is None else nb_limit
        for i in range(nb):
            Si = int(S[i])
            g = gt.tile([P, Si, ROWB], bf16, tag="gath")
            for c0 in range(0, Si, SMAXG):
                Sc = min(SMAXG, Si - c0)
                nc.gpsimd.dma_gather(
                    out_ap=g[:, c0:c0 + Sc, :],
                    in_ap=table[:],
                    idxs_ap=idx_sb[:, off:off + 8 * Sc],
                    num_idxs=P * Sc,
                    num_idxs_reg=P * Sc,
                    elem_size=ROWB,
                )
                off += 8 * Sc
            if not do_edge:
                nc.sync.dma_start(out=outbuf[i * P:(i + 1) * P, :HD // 2],
                                  in_=g[:, 0, :HD].bitcast(f32))
                continue

            gf = g[:].bitcast(f32)                          # [P, Si, 192]
            el = gf[:, :Si - 1, 128:128 + H]                # [P, Si-1, 4]
            er = gf[:, Si - 1:Si, 128 + H:128 + 2 * H].to_broadcast(
                [P, Si - 1, H])
            lg = sb.tile([P, (Si - 1) * H], f32, tag="lg")
            lg3 = lg[:].rearrange("p (s h) -> p s h", h=H)
            nc.vector.tensor_tensor(out=lg3, in0=el, in1=er, op=Alu.add)
            nc.vector.scalar_tensor_tensor(out=lg3, in0=lg3, scalar=NEG_ATTN,
                                           in1=lg3, op0=Alu.mult, op1=Alu.max)
            ex = sb.tile([P, (Si - 1) * H], bf16, tag="ex")
            nc.scalar.activation(out=ex[:], in_=lg[:], func=Act.Exp)

            den = sb.tile([P, H], f32, tag="den")
            nc.vector.tensor_reduce(
                out=den[:], in_=ex[:].rearrange("p (s h) -> p h s", h=H),
                axis=mybir.AxisListType.X, op=Alu.add)
            rec = sb.tile([P, H], f32, tag="rec")
            nc.vector.reciprocal(out=rec[:], in_=den[:])

            prod = pr.tile([P, Si - 1, HD], bf16, tag="prod")
            feat4 = g[:, :Si - 1, :HD].rearrange("p s (h d) -> p s h d", h=H)
            ex4 = ex[:].rearrange("p (s h) -> p s h", h=H).unsqueeze(3) \
                .to_broadcast([P, Si - 1, H, D])
            nc.vector.tensor_tensor(
                out=prod[:].rearrange("p s (h d) -> p s h d", h=H),
                in0=feat4, in1=ex4, op=Alu.mult)

            z = sb.tile([P, HD], f32, tag="z")
            nc.vector.tensor_reduce(
                out=z[:], in_=prod[:].rearrange("p s c -> p c s"),
                axis=mybir.AxisListType.X, op=Alu.add)
            nc.vector.tensor_tensor(
                out=z[:].rearrange("p (h d) -> p h d", h=H),
                in0=z[:].rearrange("p (h d) -> p h d", h=H),
                in1=rec[:].unsqueeze(2).to_broadcast([P, H, D]), op=Alu.mult)
            nc.vector.tensor_tensor(out=z[:], in0=z[:], in1=bias_sb[:],
                                    op=Alu.add)
            nc.vector.scalar_tensor_tensor(out=z[:], in0=z[:], scalar=NEG_ACT,
                                           in1=z[:], op0=Alu.mult, op1=Alu.max)
            nc.sync.dma_start(out=outbuf[i * P:(i + 1) * P, :], in_=z[:])

            # semantic partials: scols[:, i] = sum_h tanh(z @ Wp1 + bp1) @ Wp2
            if not do_sem:
                continue
            sem_lv = 4 if do_sem is True else int(do_sem)
            h1 = ps.tile([P, 128], f32, space="PSUM", tag="h1")
            for k, wsb in ((0, Wp1a_sb), (1, Wp1b_sb)):
                pt = ps.tile([P, P], f32, space="PSUM", tag="pt")
                nc.tensor.transpose(out=pt[:], in_=z[:, k * P:(k + 1) * P],
                                    identity=ident[:])
                zt = sb.tile([P, P], f32, tag="zt")
                nc.vector.tensor_copy(out=zt[:], in_=pt[:])
                if sem_lv >= 2:
                    nc.tensor.matmul(out=h1[:], lhsT=zt[:], rhs=wsb[:],
                                     start=(k == 0), stop=(k == 1))
            if sem_lv < 2:
                nc.vector.tensor_copy(out=scols[:, i:i + 1], in_=zt[:, 0:1])
                continue
            th = sb.tile([P, 128], f32, tag="th")
            nc.vector.tensor_tensor(out=th[:], in0=h1[:], in1=bp1_sb[:],
                                    op=Alu.add)
            if sem_lv >= 3:
                nc.scalar.activation(out=th[:], in_=th[:], func=Act.Tanh)
            if sem_lv < 4:
                nc.vector.tensor_copy(out=scols[:, i:i + 1], in_=th[:, 0:1])
                continue
            ttr = sb.tile([P, 128], f32, tag="ttr")
            nc.vector.tensor_tensor(out=ttr[:], in0=th[:], in1=Wp2_sb[:],
                                    op=Alu.mult)
            nc.vector.tensor_reduce(out=scols[:, i:i + 1], in_=ttr[:],
                                    axis=mybir.AxisListType.X, op=Alu.add)

        if do_edge and do_sem and nb == NTB:
            ssum = sb.tile([P, 1], f32, tag="ssum")
            nc.vector.tensor_reduce(out=ssum[:], in_=scols[:],
                                    axis=mybir.AxisListType.X, op=Alu.add)
            nc.sync.dma_start(out=s_out[:, :], in_=ssum[:])
        else:
            zz = sb.tile([P, 1], f32, tag="ssum")
            nc.vector.memset(zz[:], 0.0)
            nc.sync.dma_start(out=s_out[:, :], in_=zz[:])

    nc.compile()
    return nc


# ---------------------------------------------------------------- driver
def _pack_padrow():
    buf = np.zeros(768, np.uint8)
    scal = np.zeros(8, np.float32)
    scal[:H] = EL_NEG
    buf[512:544] = scal.view(np.uint8)
    return buf.view(np.uint16).reshape(1, ROWB)            # raw bf16 carrier


def _prepare(hs, src, dst, W, attn_l, attn_r, bias, Wp1, bp1, Wp2):
    import ml_dtypes

    preps = [_prep_metapath(src[m].astype(np.int64), dst[m].astype(np.int64))
             for m in range(M)]
    S = _common_schedule(preps)

    pad_u16 = _pack_padrow()
    in_maps = []
    for c in range(NCORES):
        m, h = c % M, c // M
        hsT = np.zeros((P, TROWS), ml_dtypes.bfloat16)
        hsT[:, :N] = hs[m].T.astype(ml_dtypes.bfloat16)
        Al = np.zeros((HD, H), np.float32)
        Ar = np.zeros((HD, H), np.float32)
        for hh in range(H):
            Al[hh * D:(hh + 1) * D, hh] = attn_l[m, hh]
            Ar[hh * D:(hh + 1) * D, hh] = attn_r[m, hh]
        Wxm = np.concatenate([W[m], W[m] @ Al, W[m] @ Ar], axis=1)
        in_maps.append({
            "hsT": np.ascontiguousarray(hsT),
            "Wx": Wxm.astype(ml_dtypes.bfloat16),
            "padrow": pad_u16.view(ml_dtypes.bfloat16),
            "idxs": _core_idxs(preps[m], h, S),
            "bias_rep": np.broadcast_to(bias[m], (P, HD)).astype(np.float32).copy(),
            "Wp1": Wp1.astype(np.float32),
            "bp1_rep": np.broadcast_to(bp1, (P, 128)).astype(np.float32).copy(),
            "Wp2T_rep": np.broadcast_to(Wp2[:, 0], (P, 128)).astype(np.float32).copy(),
        })
    return preps, S, in_maps


def _get_nc(S):
    key = tuple(int(x) for x in S)
    if key not in _BUILD_CACHE:
        _BUILD_CACHE[key] = _build_bass(S)
    return _BUILD_CACHE[key]


def _run_device(inputs, trace=False):
    from concourse.bass_utils import run_bass_kernel_spmd

    preps, S, in_maps = _prepare(**inputs)
    nc = _get_nc(S)
    res = run_bass_kernel_spmd(nc, in_maps, list(range(NCORES)), trace=trace)
    return preps, res


def _combine(preps, res, bias, Wp1, bp1, Wp2):
    tile_rows = np.arange(NTB)
    outs = []
    s_mean = []
    corr_z = np.where(bias > 0, bias, NEG_ACT * bias).astype(np.float64)  # [M,HD]
    for m in range(M):
        full = np.zeros((N, HD), np.float32)
        s_tot = 0.0
        for h in range(2):
            r = res.results[m + 4 * h]
            slot_pos = ((2 * tile_rows + h)[:, None] * P + np.arange(P)).ravel()
            nodes = preps[m]["slot_nodes"][slot_pos]
            mask = nodes >= 0
            full[nodes[mask]] = r["out"][mask]
            s_tot += float(r["s_out"].sum())
        corr = (np.tanh(corr_z[m] @ Wp1 + bp1) @ Wp2).item()
        s_mean.append((s_tot - NDUMMY * corr) / N)
        outs.append(full)

    def blend(pair):
        w = np.array([s_mean[pair[0]], s_mean[pair[1]]], np.float64)
        w -= w.max()
        beta = np.exp(w) / np.exp(w).sum()
        return (beta[0] * outs[pair[0]] + beta[1] * outs[pair[1]]).astype(np.float32)

    return np.stack([blend((1, 2)), blend((0, 3))])


def kernel(hs, src, dst, W, attn_l, attn_r, bias, Wp1, bp1, Wp2):
    inputs = {
        "hs": np.asarray(hs, np.float32), "src": np.asarray(src),
        "dst": np.asarray(dst), "W": np.asarray(W, np.float32),
        "attn_l": np.asarray(attn_l, np.float32),
        "attn_r": np.asarray(attn_r, np.float32),
        "bias": np.asarray(bias, np.float32),
        "Wp1": np.asarray(Wp1, np.float32),
        "bp1": np.asarray(bp1, np.float32),
        "Wp2": np.asarray(Wp2, np.float32),
    }
    preps, res = _run_device(inputs)
    return _combine(preps, res, inputs["bias"], inputs["Wp1"],
                    inputs["bp1"], inputs["Wp2"])


# revision 28
# speedup vs baseline: 4129.0385x; 1.3975x over previous
"""HAN layer (4-metapath GAT + semantic attention) on Trainium2, 8 NeuronCores.

Sharding: core c -> metapath m = c % 4, node-half h = c // 4.
Per metapath, nodes are sorted by in-degree (ascending, 208 dummy slots first)
into 236 tiles of 128; half h takes tiles g = 2i + h (118 tiles/core), so both
halves see near-identical degree profiles and one SPMD program (common slot
schedule S[i] = max over the 8 shards) fits all cores with ~no padding waste.

Device pipeline per core:
  Phase A: feat = hs @ W on the tensor engine -> gather table
           [30081 x 512B] bf16 rows (row 30080 = zero pad row).
  Phase B: per node tile, dma_gather pulls S slot-columns x 128 feat rows
           (<=8 columns per call: the SWDGE ring dies above 1024 idxs);
           attention logits el[src]+er[dst] use host-computed el/er scalars
           (el ships pre-padded per slot, er per node); DVE/ACT compute
           leaky->exp->normalize->aggregate; per-tile semantic partials
           tanh(z@Wp1+bp1)@Wp2 accumulate via PE transpose + matmul.
Host: el/er projection (120 MFLOP), index prep, final beta softmax + blend.
"""
import sys
import numpy as np

sys.path.insert(0, "/opt/trn_rl_repo")

N, E, IN, H, D = 30000, 300000, 128, 4, 64
HD = H * D                      # 256
M = 4
NCORES = 8
P = 128
NPAD = 30208                    # 236 * 128 node slots per metapath
NTG = NPAD // P                 # 236 global node tiles
NTB = NTG // 2                  # 118 tiles per core
NDUMMY = NPAD - N               # 208
TROWS = 30080                   # 235*128 real table rows (>= N)
NTA = TROWS // P                # 235 phase-A tiles
PADIDX = 30080                  # pad row index in table
TBL_ROWS = TROWS + 1            # 30081
ROWB = 256                      # bf16 elems per table row (512 B)
OUTROWS = NTB * P               # 15104
NEG_ATTN = 0.2
NEG_ACT = 0.01
EL_NEG = -300.0                 # pad-slot el: exp(0.2*(-300+er)) ~ 1e-26
SMAXG = 8                       # dma_gather dies above 1024 idxs => <=8 cols

_BUILD_CACHE = {}


# ---------------------------------------------------------------- host: indices
def _prep_metapath(src_m, dst_m):
    """Degree-sorted slot layout + padded neighbor matrix for one metapath."""
    deg = np.bincount(dst_m, minlength=N)
    order_nodes = np.argsort(deg, kind="stable")          # ascending degree
    slot_nodes = np.concatenate([np.full(NDUMMY, -1, np.int64), order_nodes])
    slot_of = np.empty(N, np.int64)
    slot_of[order_nodes] = np.arange(NDUMMY, NPAD)

    eorder = np.argsort(dst_m, kind="stable")
    ds = dst_m[eorder]
    ss = src_m[eorder]
    starts = np.concatenate([[0], np.cumsum(deg)])[:N]
    within = np.arange(E) - starts[ds]

    maxdeg = int(deg.max())
    A = np.full((NPAD, maxdeg), PADIDX, np.int16)
    A[slot_of[ds], within] = ss.astype(np.int16)

    degs_sorted = np.concatenate([np.zeros(NDUMMY, np.int64), deg[order_nodes]])
    tiledeg = degs_sorted.reshape(NTG, P).max(axis=1)     # [236]
    return {"A": A, "tiledeg": tiledeg, "slot_nodes": slot_nodes}


def _common_schedule(preps):
    """S[i] edge slots per core-tile rank i (shared across all 8 cores)."""
    S = np.zeros(NTB, np.int64)
    for i in range(NTB):
        md = 0
        for p in preps:
            md = max(md, int(p["tiledeg"][2 * i]), int(p["tiledeg"][2 * i + 1]))
        S[i] = md
    return np.maximum(S, 1)


def _core_tables(prep, h, S, el_full, er_full):
    """Per-core gather indices (int16 wrap), el slot values, er node values."""
    idx_blocks = []
    el_cols = []
    er_cols = np.zeros((P, NTB * H), np.float32)
    el_ext = np.concatenate(
        [el_full, np.zeros((TBL_ROWS - N, H), np.float32)], axis=0)
    for i in range(NTB):
        g = 2 * i + h
        rows = slice(g * P, (g + 1) * P)
        Si = int(S[i])
        full = np.full((P, Si), PADIDX, np.int16)
        width = min(Si, prep["A"].shape[1])
        full[:, :width] = prep["A"][rows, :width]
        for c0 in range(0, Si, SMAXG):                     # <=8-col chunks
            Sc = min(SMAXG, Si - c0)
            T = full[:, c0:c0 + Sc].T.ravel()              # e = s*128 + p
            idx_blocks.append(T.reshape(8 * Sc, 16).T)     # [16, 8*Sc]
        elv = el_ext[full.astype(np.int64)]                # [P, Si, H]
        elv[full == PADIDX] = EL_NEG
        el_cols.append(elv.reshape(P, Si * H).astype(np.float32))
        nodes = prep["slot_nodes"][rows]
        erv = np.where(nodes[:, None] >= 0,
                       er_full[np.maximum(nodes, 0)], 0.0)
        er_cols[:, i * H:(i + 1) * H] = erv
    cols = np.concatenate(idx_blocks, axis=1)
    idxs = np.tile(cols, (8, 1))    # 16-partition wrap x 8 Q7-core groups
    return idxs, np.concatenate(el_cols, axis=1), er_cols


# ---------------------------------------------------------------- device build
def _build_bass(S, nb_limit=None, do_edge=True, do_sem=True):
    import concourse.bacc as bacc
    import concourse.tile as tile
    from concourse import mybir
    from concourse.masks import make_identity
    from contextlib import ExitStack

    f32 = mybir.dt.float32
    bf16 = mybir.dt.bfloat16
    i16 = mybir.dt.int16
    Alu = mybir.AluOpType
    Act = mybir.ActivationFunctionType
    IC = int(8 * np.sum(S))
    ELC = int(H * np.sum(S))

    nc = bacc.Bacc()
    hsT = nc.declare_dram_parameter("hsT", (P, TROWS), bf16, isOutput=False)
    Wd = nc.declare_dram_parameter("W", (P, HD), bf16, isOutput=False)
    padrow = nc.declare_dram_parameter("padrow", (1, ROWB), bf16, isOutput=False)
    idxs_d = nc.declare_dram_parameter("idxs", (P, IC), i16, isOutput=False)
    el_d = nc.declare_dram_parameter("el", (P, ELC), f32, isOutput=False)
    er_d = nc.declare_dram_parameter("er", (P, NTB * H), f32, isOutput=False)
    bias_rep = nc.declare_dram_parameter("bias_rep", (P, HD), f32, isOutput=False)
    Wp1_d = nc.declare_dram_parameter("Wp1", (HD, 128), f32, isOutput=False)
    bp1_rep = nc.declare_dram_parameter("bp1_rep", (P, 128), f32, isOutput=False)
    Wp2T_rep = nc.declare_dram_parameter("Wp2T_rep", (P, 128), f32, isOutput=False)
    outbuf = nc.declare_dram_parameter("out", (OUTROWS, HD), f32, isOutput=True)
    s_out = nc.declare_dram_parameter("s_out", (P, 1), f32, isOutput=True)

    with tile.TileContext(nc) as tc, ExitStack() as ctx:
        big = ctx.enter_context(tc.tile_pool(name="big", bufs=1))
        sb = ctx.enter_context(tc.tile_pool(name="sb", bufs=3))
        gt = ctx.enter_context(tc.tile_pool(name="gt", bufs=3))
        pr = ctx.enter_context(tc.tile_pool(name="pr", bufs=2))
        ps = ctx.enter_context(tc.tile_pool(name="ps", bufs=2, space="PSUM"))
        dram = ctx.enter_context(tc.tile_pool(name="dram", bufs=1, space="DRAM"))

        table = dram.tile([TBL_ROWS, ROWB], bf16, tag="table")

        # ---- resident loads
        hsT_sb = big.tile([P, TROWS], bf16, tag="hsT")
        # single >= ~60KB/partition DMAs fail (NRT unrecoverable); chunk it
        for q in range(4):
            lo, hi = q * (TROWS // 4), (q + 1) * (TROWS // 4)
            nc.sync.dma_start(out=hsT_sb[:, lo:hi], in_=hsT[:, lo:hi])
        W_sb = big.tile([P, HD], bf16, tag="W")
        nc.sync.dma_start(out=W_sb[:], in_=Wd[:, :])
        idx_sb = big.tile([P, IC], i16, tag="idx")
        nc.sync.dma_start(out=idx_sb[:], in_=idxs_d[:, :])
        er_sb = big.tile([P, NTB * H], f32, tag="er")
        nc.sync.dma_start(out=er_sb[:], in_=er_d[:, :])
        bias_sb = big.tile([P, HD], f32, tag="bias")
        nc.sync.dma_start(out=bias_sb[:], in_=bias_rep[:, :])
        Wp1a_sb = big.tile([P, 128], f32, tag="wp1a")
        nc.sync.dma_start(out=Wp1a_sb[:], in_=Wp1_d[0:128, :])
        Wp1b_sb = big.tile([P, 128], f32, tag="wp1b")
        nc.sync.dma_start(out=Wp1b_sb[:], in_=Wp1_d[128:256, :])
        bp1_sb = big.tile([P, 128], f32, tag="bp1")
        nc.sync.dma_start(out=bp1_sb[:], in_=bp1_rep[:, :])
        Wp2_sb = big.tile([P, 128], f32, tag="wp2")
        nc.sync.dma_start(out=Wp2_sb[:], in_=Wp2T_rep[:, :])
        ident = big.tile([P, P], f32, tag="ident")
        make_identity(nc, ident[:])
        if do_edge and do_sem:
            scols = big.tile([P, NTB], f32, tag="scols")
        else:
            scols = None

        pr_sb = sb.tile([1, ROWB], bf16, tag="padrow")
        nc.sync.dma_start(out=pr_sb[:], in_=padrow[:, :])
        nc.sync.dma_start(out=table[TROWS:TROWS + 1, :], in_=pr_sb[:])

        # ---- phase A: build gather table (feat = hs @ W, bf16 rows)
        for t in range(NTA):
            psf = ps.tile([P, HD], f32, space="PSUM", tag="psf")
            nc.tensor.matmul(out=psf[:], lhsT=hsT_sb[:, t * P:(t + 1) * P],
                             rhs=W_sb[:], start=True, stop=True)
            stage = sb.tile([P, ROWB], bf16, tag="stage")
            nc.vector.tensor_copy(out=stage[:], in_=psf[:])
            nc.sync.dma_start(out=table[t * P:(t + 1) * P, :], in_=stage[:])

        # ---- phase B: gather + edge softmax + aggregate + semantic partials
        off = 0
        eoff = 0
        nb = NTB if nb_limit is None else nb_limit
        for i in range(nb):
            Si = int(S[i])
            g = gt.tile([P, Si, ROWB], bf16, tag="gath")
            for c0 in range(0, Si, SMAXG):
                Sc = min(SMAXG, Si - c0)
                nc.gpsimd.dma_gather(
                    out_ap=g[:, c0:c0 + Sc, :],
                    in_ap=table[:],
                    idxs_ap=idx_sb[:, off:off + 8 * Sc],
                    num_idxs=P * Sc,
                    num_idxs_reg=P * Sc,
                    elem_size=ROWB,
                )
                off += 8 * Sc
            el = sb.tile([P, Si * H], f32, tag="el")
            nc.sync.dma_start(out=el[:], in_=el_d[:, eoff:eoff + Si * H])
            eoff += Si * H
            if not do_edge:
                nc.sync.dma_start(out=outbuf[i * P:(i + 1) * P, :HD // 2],
                                  in_=g[:, 0, :].bitcast(f32))
                continue

            er_b = er_sb[:, i * H:(i + 1) * H].unsqueeze(1).to_broadcast(
                [P, Si, H])
            lg = sb.tile([P, Si * H], f32, tag="lg")
            lg3 = lg[:].rearrange("p (s h) -> p s h", h=H)
            nc.vector.tensor_tensor(
                out=lg3, in0=el[:].rearrange("p (s h) -> p s h", h=H),
                in1=er_b, op=Alu.add)
            nc.vector.scalar_tensor_tensor(out=lg3, in0=lg3, scalar=NEG_ATTN,
                                           in1=lg3, op0=Alu.mult, op1=Alu.max)
            ex = sb.tile([P, Si * H], bf16, tag="ex")
            nc.scalar.activation(out=ex[:], in_=lg[:], func=Act.Exp)

            den = sb.tile([P, H], f32, tag="den")
            nc.vector.tensor_reduce(
                out=den[:], in_=ex[:].rearrange("p (s h) -> p h s", h=H),
                axis=mybir.AxisListType.X, op=Alu.add)
            rec = sb.tile([P, H], f32, tag="rec")
            nc.vector.reciprocal(out=rec[:], in_=den[:])

            prod = pr.tile([P, Si, HD], bf16, tag="prod")
            feat4 = g[:].rearrange("p s (h d) -> p s h d", h=H)
            ex4 = ex[:].rearrange("p (s h) -> p s h", h=H).unsqueeze(3) \
                .to_broadcast([P, Si, H, D])
            nc.vector.tensor_tensor(
                out=prod[:].rearrange("p s (h d) -> p s h d", h=H),
                in0=feat4, in1=ex4, op=Alu.mult)

            z = sb.tile([P, HD], f32, tag="z")
            nc.vector.tensor_reduce(
                out=z[:], in_=prod[:].rearrange("p s c -> p c s"),
                axis=mybir.AxisListType.X, op=Alu.add)
            nc.vector.tensor_tensor(
                out=z[:].rearrange("p (h d) -> p h d", h=H),
                in0=z[:].rearrange("p (h d) -> p h d", h=H),
                in1=rec[:].unsqueeze(2).to_broadcast([P, H, D]), op=Alu.mult)
            nc.vector.tensor_tensor(out=z[:], in0=z[:], in1=bias_sb[:],
                                    op=Alu.add)
            nc.vector.scalar_tensor_tensor(out=z[:], in0=z[:], scalar=NEG_ACT,
                                           in1=z[:], op0=Alu.mult, op1=Alu.max)
            nc.sync.dma_start(out=outbuf[i * P:(i + 1) * P, :], in_=z[:])

            # semantic partials: scols[:, i] = sum_h tanh(z @ Wp1 + bp1) @ Wp2
            if not do_sem:
                continue
            h1 = ps.tile([P, 128], f32, space="PSUM", tag="h1")
            for k, wsb in ((0, Wp1a_sb), (1, Wp1b_sb)):
                pt = ps.tile([P, P], f32, space="PSUM", tag="pt")
                nc.tensor.transpose(out=pt[:], in_=z[:, k * P:(k + 1) * P],
                                    identity=ident[:])
                zt = sb.tile([P, P], f32, tag="zt")
                nc.scalar.copy(out=zt[:], in_=pt[:])
                nc.tensor.matmul(out=h1[:], lhsT=zt[:], rhs=wsb[:],
                                 start=(k == 0), stop=(k == 1))
            th = sb.tile([P, 128], f32, tag="th")
            nc.vector.tensor_tensor(out=th[:], in0=h1[:], in1=bp1_sb[:],
                                    op=Alu.add)
            nc.scalar.activation(out=th[:], in_=th[:], func=Act.Tanh)
            ttr = sb.tile([P, 128], f32, tag="ttr")
            nc.vector.tensor_tensor(out=ttr[:], in0=th[:], in1=Wp2_sb[:],
                                    op=Alu.mult)
            nc.vector.tensor_reduce(out=scols[:, i:i + 1], in_=ttr[:],
                                    axis=mybir.AxisListType.X, op=Alu.add)

        if do_edge and do_sem and nb == NTB:
            ssum = sb.tile([P, 1], f32, tag="ssum")
            nc.vector.tensor_reduce(out=ssum[:], in_=scols[:],
                                    axis=mybir.AxisListType.X, op=Alu.add)
            nc.sync.dma_start(out=s_out[:, :], in_=ssum[:])
        else:
            zz = sb.tile([P, 1], f32, tag="ssum")
            nc.vector.memset(zz[:], 0.0)
            nc.sync.dma_start(out=s_out[:, :], in_=zz[:])

    nc.compile()
    return nc


# ---------------------------------------------------------------- driver
def _prepare(hs, src, dst, W, attn_l, attn_r, bias, Wp1, bp1, Wp2):
    import ml_dtypes

    preps = [_prep_metapath(src[m].astype(np.int64), dst[m].astype(np.int64))
             for m in range(M)]
    S = _common_schedule(preps)

    pad_u16 = np.zeros((1, ROWB), np.uint16)
    in_maps = []
    els = {}
    for m in range(M):
        Al = np.zeros((HD, H), np.float32)
        Ar = np.zeros((HD, H), np.float32)
        for hh in range(H):
            Al[hh * D:(hh + 1) * D, hh] = attn_l[m, hh]
            Ar[hh * D:(hh + 1) * D, hh] = attn_r[m, hh]
        WAl = W[m].astype(np.float64) @ Al.astype(np.float64)
        WAr = W[m].astype(np.float64) @ Ar.astype(np.float64)
        el_full = (hs[m].astype(np.float64) @ WAl).astype(np.float32)
        er_full = (hs[m].astype(np.float64) @ WAr).astype(np.float32)
        els[m] = (el_full, er_full)
    for c in range(NCORES):
        m, h = c % M, c // M
        hsT = np.zeros((P, TROWS), ml_dtypes.bfloat16)
        hsT[:, :N] = hs[m].T.astype(ml_dtypes.bfloat16)
        idxs, el_cols, er_cols = _core_tables(preps[m], h, S, *els[m])
        in_maps.append({
            "hsT": np.ascontiguousarray(hsT),
            "W": W[m].astype(ml_dtypes.bfloat16),
            "padrow": pad_u16.view(ml_dtypes.bfloat16),
            "idxs": idxs,
            "el": np.ascontiguousarray(el_cols),
            "er": np.ascontiguousarray(er_cols),
            "bias_rep": np.broadcast_to(bias[m], (P, HD)).astype(np.float32).copy(),
            "Wp1": Wp1.astype(np.float32),
            "bp1_rep": np.broadcast_to(bp1, (P, 128)).astype(np.float32).copy(),
            "Wp2T_rep": np.broadcast_to(Wp2[:, 0], (P, 128)).astype(np.float32).copy(),
        })
    return preps, S, in_maps


def _get_nc(S):
    key = tuple(int(x) for x in S)
    if key not in _BUILD_CACHE:
        _BUILD_CACHE[key] = _build_bass(S)
    return _BUILD_CACHE[key]


def _run_device(inputs, trace=False):
    from concourse.bass_utils import run_bass_kernel_spmd

    preps, S, in_maps = _prepare(**inputs)
    nc = _get_nc(S)
    res = run_bass_kernel_spmd(nc, in_maps, list(range(NCORES)), trace=trace)
    return preps, res


def _combine(preps, res, bias, Wp1, bp1, Wp2):
    tile_rows = np.arange(NTB)
    outs = []
    s_mean = []
    corr_z = np.where(bias > 0, bias, NEG_ACT * bias).astype(np.float64)  # [M,HD]
    for m in range(M):
        full = np.zeros((N, HD), np.float32)
        s_tot = 0.0
        for h in range(2):
            r = res.results[m + 4 * h]
            slot_pos = ((2 * tile_rows + h)[:, None] * P + np.arange(P)).ravel()
            nodes = preps[m]["slot_nodes"][slot_pos]
            mask = nodes >= 0
            full[nodes[mask]] = r["out"][mask]
            s_tot += float(r["s_out"].sum())
        corr = (np.tanh(corr_z[m] @ Wp1 + bp1) @ Wp2).item()
        s_mean.append((s_tot - NDUMMY * corr) / N)
        outs.append(full)

    def blend(pair):
        w = np.array([s_mean[pair[0]], s_mean[pair[1]]], np.float64)
        w -= w.max()
        beta = np.exp(w) / np.exp(w).sum()
        return (beta[0] * outs[pair[0]] + beta[1] * outs[pair[1]]).astype(np.float32)

    return np.stack([blend((1, 2)), blend((0, 3))])


def kernel(hs, src, dst, W, attn_l, attn_r, bias, Wp1, bp1, Wp2):
    inputs = {
        "hs": np.asarray(hs, np.float32), "src": np.asarray(src),
        "dst": np.asarray(dst), "W": np.asarray(W, np.float32),
        "attn_l": np.asarray(attn_l, np.float32),
        "attn_r": np.asarray(attn_r, np.float32),
        "bias": np.asarray(bias, np.float32),
        "Wp1": np.asarray(Wp1, np.float32),
        "bp1": np.asarray(bp1, np.float32),
        "Wp2": np.asarray(Wp2, np.float32),
    }
    preps, res = _run_device(inputs)
    return _combine(preps, res, inputs["bias"], inputs["Wp1"],
                    inputs["bp1"], inputs["Wp2"])
